# revision 24
# baseline (speedup 1.0000x reference)
"""Bahdanau additive-attention pooling for Trainium2 (Bass/Tile).

Reference math (per batch):
    q = x @ Wt; k = x @ Wx                                  [L, U]
    e[i,j] = sum_u Wa[u] * tanh(q[i,u] + k[j,u] + bh[u])    (+ ba, dropped --
                                                             softmax shift-inv)
    v = softmax_j(e) @ x                                    [L, D]

Key trick: tanh(a+b) is approximated by a short sine expansion with FITTED
frequencies (weighted nonlinear LSQ over the data distribution of s = a+b):
    tanh(s) ~= sum_j b_j sin(lam_j s),   j = 1..6
which is SEPARABLE:
    sin(lam(a+b)) = sin(lam a)cos(lam b) + cos(lam a)sin(lam b)
so the [L, L, U] tanh volume (the baseline bottleneck: ~131us of ScalarE at
1 elem/cycle/lane) collapses into a rank-12-per-u PE matmul:
    e[i,j] = sum_{u,f} F[(u,f), i] * G[(u,f), j]
    F = Wa_u b_j sin(lam_f q_iu + phi_f),  G = sin(lam_f k_ju + psi_f)
with NF = 12 features -> 32*12 = 384 contraction = 3 chunks of 128.

The HW Sin activation table is only accurate for |arg| <= ~pi, so arguments
are range-reduced first.  MOD/floor do not exist on DVE, but fp32->int
convert-on-write rounds-to-nearest (verified on HW for both DVE and GPSIMD),
so per feature f:
    t  = head * (lam_f/2pi)            (head = q or k, SBUF fp32)
    nf = int32(t + c_f)                 # GPSIMD/DVE tensor_scalar (mult, add)
    r  = t - nf                         # DVE scalar_tensor_tensor, fp32
    out = Sin(2pi * r + 2pi*c_f)        # ACT, per-partition bias, |arg|<=pi
      == sin(lam_f head + phi_f)        exactly (c_f = phi_f/2pi + bh lam/2pi)

Sharding: 8 cores = 4 batches x 2 query-halves (data-parallel, no
collectives).  Host rotates x per core so queries are always rows 0..511
(softmax over keys is order-invariant, so the rotated key order is fine).

Per-core pipeline (512 q x 1024 k):
  x -> xT (PE transposes) -> qT4/kT4 4x-replicated heads (PE matmul,
  partitions = 4 features x 32 u) -> heads to SBUF (ACT copies) ->
  range-reduce (GPSIMD p1 + DVE p2, chunk-pipelined) -> ACT Sin basis
  passes (per-partition bias; fp16 out) -> wco fold (DVE, 4x fp16) ->
  e = F^T G (PE, 3x2 accumulating matmuls per 128-query block) -> exp on ACT
  w/ accum_out row sums -> PE transpose (fp16) -> v = a^T @ x16 (PE) ->
  scale by 1/rowsum (DVE) -> DMA out.  Tails are staggered one block behind
  the e-matmuls; the last block's tail is split into key-halves to shorten
  the final serial chain.  A dummy Sin pins the sin activation table from
  t=0 (avoids a 1.3us mid-pipeline table swap), and ~14 filler matmuls keep
  the PE p-state ramped between the heads and the first e-matmul.
"""

import numpy as np

import concourse.bass as bass
import concourse.mybir as mybir
import concourse.tile as tile
from concourse import bacc
from concourse.bass import ds, ts

B, L, D, U = 4, 1024, 256, 32
NCORES = 8
HALVES = 2
LQ = L // HALVES                # 512 queries per core
NDC = D // 128                  # 2 contraction chunks for q/k projections
NJC = L // 128                  # 8 key chunks
NIB = LQ // 128                 # 4 query blocks
NFREQ = 6                       # fitted sine frequencies
NF = 2 * NFREQ                  # basis features (sin+cos per frequency)
NCH = NF // 4                   # 3 contraction chunks of 128 partitions
TWO_PI = float(2 * np.pi)

# Weighted nonlinear LSQ fit of tanh(s) ~= sum_j BCO[j] sin(LAM[j] s) over
# s in [-10, 10], weight exp(-s^2/11.5)+0.02 (s = q+k is approx N(0, 1.45^2)).
# End-to-end output rel err ~9e-4 (gate: 2e-3 local, 2e-2 harness).
LAM = [0.2781557257, 0.8391562854, 1.4111810023,
       2.0111781003, 2.7125102019, 3.6320141768]
BCO = [1.2355136439, 0.3262321042, 0.1278783401,
       0.0538655712, 0.0222520230, 0.0071252936]

F32 = mybir.dt.float32
F32R = mybir.dt.float32r
F16 = mybir.dt.float16
I32 = mybir.dt.int32
AF = mybir.ActivationFunctionType
ALU = mybir.AluOpType

# tabs columns: inv_per, c_q, c_k, bias_q (=2pi c_q), bias_k, wco
TINV, TCQ, TCK, TBQ, TBK, TWCO = range(6)


def build_kernel(nc: bass.Bass):
    xt_d = nc.dram_tensor("xt", [NDC, 128, L], F32R, kind="ExternalInput")
    x16_d = nc.dram_tensor("x16", [L, D], F16, kind="ExternalInput")
    wt4_d = nc.dram_tensor("wt4", [NDC, 128, 128], F32R, kind="ExternalInput")
    wx4_d = nc.dram_tensor("wx4", [NDC, 128, 128], F32R, kind="ExternalInput")
    tabs_d = nc.dram_tensor("tabs", [128, NCH, 6], F32, kind="ExternalInput")
    id16_d = nc.dram_tensor("id16", [128, 128], F16, kind="ExternalInput")
    out_d = nc.dram_tensor("out", [LQ, D], F32, kind="ExternalOutput")

    with tile.TileContext(nc) as tc:
        with tc.tile_pool(name="const", bufs=1) as cpool:
            x16_sb = cpool.tile([128, NJC, D], F16)
            xT_sb = cpool.tile([128, NDC, L], F32R)
            wt4_sb = cpool.tile([128, NDC, 128], F32R)
            wx4_sb = cpool.tile([128, NDC, 128], F32R)
            tabs_sb = cpool.tile([128, NCH, 6], F32)
            id16_sb = cpool.tile([128, 128], F16)
            qts_sb = cpool.tile([128, LQ], F32)
            kts_sb = cpool.tile([128, L], F32)
            msq_sb = cpool.tile([128, NCH, LQ], F32)
            msk_sb = cpool.tile([128, NCH, L], F32)
            ft_sb = cpool.tile([128, NCH, LQ], F16)
            gt_sb = cpool.tile([128, NCH, L], F16)
            sums_sb = cpool.tile([128, NIB, 2], F32)
            recip_sb = cpool.tile([128, NIB], F32)
            scr_sb = cpool.tile([128, 1], F16)

            # DMA order: weights first (gate the PE warm-up fillers and
            # heads), then xT in key-halves so kT4 n=0 can start early.
            xt_r = xt_d.ap().rearrange("c p j -> p c j")
            nc.sync.dma_start(xT_sb[:, :, ds(0, 512)], xt_r[:, :, 0:512])
            nc.sync.dma_start(
                wt4_sb[:], wt4_d.ap().rearrange("c p m -> p c m")
            )
            nc.sync.dma_start(
                wx4_sb[:], wx4_d.ap().rearrange("c p m -> p c m")
            )
            nc.sync.dma_start(xT_sb[:, :, ds(512, 512)], xt_r[:, :, 512:])
            nc.scalar.dma_start(tabs_sb[:], tabs_d.ap())
            nc.scalar.dma_start(id16_sb[:], id16_d.ap())
            nc.gpsimd.dma_start(
                x16_sb[:], x16_d.ap().rearrange("(c p) d -> p c d", p=128)
            )

            # Dummy Sin: pins the sin-containing activation table from the
            # start so no table swap lands on the basis-pass critical path.
            nc.scalar.activation(scr_sb[:], tabs_sb[:, 0, ds(0, 1)], AF.Sin)

            # ---- prologue: xT, q/k heads -> SBUF ----
            with (
                tc.tile_pool(name="pk", bufs=1, space="PSUM") as pk,
                tc.tile_pool(name="ptr", bufs=2, space="PSUM") as ptr,
                tc.tile_pool(name="pq", bufs=1, space="PSUM") as pq,
            ):
                kt_ps = pk.tile([128, L], F32)
                qt_ps = pq.tile([128, LQ], F32)
                # PE p-state warm-up: fillers gated only on wt4 (the first
                # DMA) so the tensor engine ramps before the heads arrive,
                # and interleaved with the heads to bridge to the e-matmuls.
                scr_ps = ptr.tile([128, 512], F32)

                def filler(count):
                    for dmy in range(count):
                        nc.tensor.matmul(
                            scr_ps[:, ds(0, 128)],
                            wt4_sb[:, 0, :],
                            wt4_sb[:, 0, :],
                            start=True,
                            stop=True,
                        )

                filler(5)
                for n in range(L // 512):
                    for dc in range(NDC):
                        nc.tensor.matmul(
                            kt_ps[:, ds(n * 512, 512)],
                            wx4_sb[:, dc, :],
                            xT_sb[:, dc, ds(n * 512, 512)],
                            start=(dc == 0),
                            stop=(dc == NDC - 1),
                        )
                    filler(2)
                for dc in range(NDC):
                    nc.tensor.matmul(
                        qt_ps[:],
                        wt4_sb[:, dc, :],
                        xT_sb[:, dc, ds(0, LQ)],
                        start=(dc == 0),
                        stop=(dc == NDC - 1),
                    )
                filler(10)
                nc.scalar.copy(kts_sb[:, ds(0, 512)], kt_ps[:, ds(0, 512)])
                nc.scalar.copy(kts_sb[:, ds(512, 512)], kt_ps[:, ds(512, 512)])
                nc.scalar.copy(qts_sb[:], qt_ps[:])

            # ---- chunk-pipelined range reduction + basis ----
            with (
                tc.tile_pool(name="nfq", bufs=2) as nfqp,
                tc.tile_pool(name="nfk", bufs=3) as nfkp,
                tc.tile_pool(name="spool", bufs=2) as spool,
            ):
                for c in range(NCH):
                    for nh in range(2):
                        nfk = nfkp.tile([128, 512], I32)
                        nc.gpsimd.tensor_scalar(
                            nfk[:],
                            kts_sb[:, ds(nh * 512, 512)],
                            tabs_sb[:, c, ds(TINV, 1)],
                            tabs_sb[:, c, ds(TCK, 1)],
                            ALU.mult,
                            ALU.add,
                        )
                        nc.vector.scalar_tensor_tensor(
                            msk_sb[:, c, ds(nh * 512, 512)],
                            kts_sb[:, ds(nh * 512, 512)],
                            tabs_sb[:, c, ds(TINV, 1)],
                            nfk[:],
                            ALU.mult,
                            ALU.subtract,
                        )
                    nfq = nfqp.tile([128, LQ], I32)
                    # q-side p1 on GPSIMD except the last chunk (balance)
                    p1q = nc.gpsimd if c < NCH - 1 else nc.vector
                    p1q.tensor_scalar(
                        nfq[:],
                        qts_sb[:],
                        tabs_sb[:, c, ds(TINV, 1)],
                        tabs_sb[:, c, ds(TCQ, 1)],
                        ALU.mult,
                        ALU.add,
                    )
                    nc.vector.scalar_tensor_tensor(
                        msq_sb[:, c, :],
                        qts_sb[:],
                        tabs_sb[:, c, ds(TINV, 1)],
                        nfq[:],
                        ALU.mult,
                        ALU.subtract,
                    )
                    ftm = spool.tile([128, LQ], F16, tag="ftm")
                    nc.scalar.activation(
                        ftm[:],
                        msq_sb[:, c, :],
                        AF.Sin,
                        bias=tabs_sb[:, c, ds(TBQ, 1)],
                        scale=TWO_PI,
                    )
                    nc.scalar.activation(
                        gt_sb[:, c, :],
                        msk_sb[:, c, :],
                        AF.Sin,
                        bias=tabs_sb[:, c, ds(TBK, 1)],
                        scale=TWO_PI,
                    )
                    nc.vector.tensor_scalar_mul(
                        ft_sb[:, c, :], ftm[:], tabs_sb[:, c, ds(TWCO, 1)]
                    )

                # ---- main loop over query blocks ----
                with (
                    tc.tile_pool(name="ppool", bufs=3) as ppool,
                    tc.tile_pool(name="atpool", bufs=2) as atpool,
                    tc.tile_pool(name="vpool", bufs=2) as vpool,
                    tc.tile_pool(name="pe", bufs=5, space="PSUM") as pe_e,
                    tc.tile_pool(name="pat", bufs=2, space="PSUM") as pe_at,
                    tc.tile_pool(name="pv", bufs=1, space="PSUM") as pe_v,
                ):
                    out_r = out_d.ap().rearrange("(ib p) d -> ib p d", p=128)
                    e_tiles = {}

                    def tail(ib, last=False):
                        p = ppool.tile([128, L], F16, tag="p")
                        at_ps = pe_at.tile([128, L], F16)
                        at_sb = atpool.tile([128, NJC, 128], F16, tag="at")
                        v_ps = pe_v.tile([128, D], F32)
                        hjc = NJC // 2
                        for h in (0, 1):
                            nc.scalar.activation(
                                p[:, ds(h * 512, 512)],
                                e_tiles[ib][h][:],
                                AF.Exp,
                                accum_out=sums_sb[:, ib, ds(h, 1)],
                            )
                            for jj in range(hjc):
                                jc = h * hjc + jj
                                nc.tensor.transpose(
                                    at_ps[:, ts(jc, 128)],
                                    p[:, ts(jc, 128)],
                                    id16_sb[:],
                                )
                            nc.vector.tensor_copy(
                                at_sb[:, h * hjc : (h + 1) * hjc, :],
                                at_ps[:, ds(h * 512, 512)],
                            )
                            for jj in range(hjc):
                                jc = h * hjc + jj
                                nc.tensor.matmul(
                                    v_ps[:],
                                    at_sb[:, jc, :],
                                    x16_sb[:, jc, :],
                                    start=(jc == 0),
                                    stop=(jc == NJC - 1),
                                )
                        nc.vector.tensor_tensor(
                            sums_sb[:, ib, ds(0, 1)],
                            sums_sb[:, ib, ds(0, 1)],
                            sums_sb[:, ib, ds(1, 1)],
                            ALU.add,
                        )
                        nc.vector.reciprocal(
                            recip_sb[:, ds(ib, 1)], sums_sb[:, ib, ds(0, 1)]
                        )
                        v_sb = vpool.tile([128, D], F32, tag="v")
                        nc.vector.tensor_scalar_mul(
                            v_sb[:], v_ps[:], recip_sb[:, ds(ib, 1)]
                        )
                        nc.sync.dma_start(out_r[ib], v_sb[:])

                    for ib in range(NIB):
                        e_h0 = pe_e.tile([128, 512], F32, tag="e")
                        e_h1 = pe_e.tile([128, 512], F32, tag="e")
                        e_h = [e_h0, e_h1]
                        e_tiles[ib] = e_h
                        for c in range(NCH):
                            for n in range(L // 512):
                                nc.tensor.matmul(
                                    e_h[n][:],
                                    ft_sb[:, c, ds(ib * 128, 128)],
                                    gt_sb[:, c, ds(n * 512, 512)],
                                    start=(c == 0),
                                    stop=(c == NCH - 1),
                                )
                        if ib >= 1:
                            tail(ib - 1)
                    tail(NIB - 1, last=True)

    return nc


_NC_CACHE: dict = {}


def get_compiled_nc():
    if "nc" not in _NC_CACHE:
        nc = bacc.Bacc("TRN2", target_bir_lowering=False, debug=False)
        build_kernel(nc)
        nc.compile()
        _NC_CACHE["nc"] = nc
    return _NC_CACHE["nc"]


def make_tables(bh, Wa):
    """Per-partition tables: partition p = 32*g + u holds feature f = 4*c + g
    (chunk c) for head u.  Feature f: frequency j = f//2; q-side phase
    phi = 0 (f even, sin) or pi/2 (f odd, cos); k-side phase is swapped so
    sum_f F*G telescopes to sum_j b_j sin(lam_j (a+b))."""
    tabs = np.zeros((128, NCH, 6), np.float64)
    for c in range(NCH):
        for g in range(4):
            f = 4 * c + g
            j = f // 2
            lam = LAM[j]
            phi_q = 0.0 if f % 2 == 0 else np.pi / 2
            phi_k = np.pi / 2 if f % 2 == 0 else 0.0
            for u in range(U):
                p = 32 * g + u
                cq = phi_q / (2 * np.pi) + bh[u] * lam / (2 * np.pi)
                ck = phi_k / (2 * np.pi)
                tabs[p, c, TINV] = lam / (2 * np.pi)
                tabs[p, c, TCQ] = cq
                tabs[p, c, TCK] = ck
                tabs[p, c, TBQ] = 2 * np.pi * cq
                tabs[p, c, TBK] = 2 * np.pi * ck
                tabs[p, c, TWCO] = Wa[u, 0] * BCO[j]
    return tabs.astype(np.float32)


def make_in_maps(inputs_np, Wt, Wx, bh, Wa):
    wt4 = np.zeros((NDC, 128, 128), np.float32)
    wx4 = np.zeros((NDC, 128, 128), np.float32)
    for dc in range(NDC):
        wt4[dc] = np.tile(Wt[dc * 128 : (dc + 1) * 128], (1, 4))
        wx4[dc] = np.tile(Wx[dc * 128 : (dc + 1) * 128], (1, 4))
    tabs = make_tables(bh, Wa)
    id16 = np.eye(128, dtype=np.float16)
    in_maps = []
    for core in range(NCORES):
        b, half = divmod(core, HALVES)
        xr = np.roll(inputs_np[b], -half * LQ, axis=0)
        in_maps.append(
            {
                "xt": np.ascontiguousarray(xr.T.reshape(NDC, 128, L)),
                "x16": np.ascontiguousarray(xr.astype(np.float16)),
                "wt4": wt4,
                "wx4": wx4,
                "tabs": tabs,
                "id16": id16,
            }
        )
    return in_maps


def kernel(**inputs) -> np.ndarray:
    x = np.asarray(inputs["inputs"], dtype=np.float32)
    Wt = np.ascontiguousarray(np.asarray(inputs["Wt"], np.float32))
    Wx = np.ascontiguousarray(np.asarray(inputs["Wx"], np.float32))
    bh = np.asarray(inputs["bh"], np.float32)
    Wa = np.asarray(inputs["Wa"], np.float32)

    from concourse.bass_utils import run_bass_kernel_spmd

    nc = get_compiled_nc()
    in_maps = make_in_maps(x, Wt, Wx, bh, Wa)
    res = run_bass_kernel_spmd(nc, in_maps, list(range(NCORES)))
    kernel._last_results = res  # type: ignore[attr-defined]

    out = np.empty((B, L, D), np.float32)
    for core in range(NCORES):
        b, half = divmod(core, HALVES)
        out[b, half * LQ : (half + 1) * LQ] = res.results[core]["out"]
    return out


# revision 25
# speedup vs baseline: 1.0373x; 1.0373x over previous
"""Bahdanau additive-attention pooling for Trainium2 (Bass/Tile).

Reference math (per batch):
    q = x @ Wt; k = x @ Wx                                  [L, U]
    e[i,j] = sum_u Wa[u] * tanh(q[i,u] + k[j,u] + bh[u])    (+ ba, dropped --
                                                             softmax shift-inv)
    v = softmax_j(e) @ x                                    [L, D]

Key trick: tanh(a+b) is approximated by a short sine expansion with FITTED
frequencies (weighted nonlinear LSQ over the data distribution of s = a+b):
    tanh(s) ~= sum_j b_j sin(lam_j s),   j = 1..6
which is SEPARABLE:
    sin(lam(a+b)) = sin(lam a)cos(lam b) + cos(lam a)sin(lam b)
so the [L, L, U] tanh volume (the baseline bottleneck: ~131us of ScalarE at
1 elem/cycle/lane) collapses into a rank-12-per-u PE matmul:
    e[i,j] = sum_{u,f} F[(u,f), i] * G[(u,f), j]
    F = Wa_u b_j sin(lam_f q_iu + phi_f),  G = sin(lam_f k_ju + psi_f)
with NF = 12 features -> 32*12 = 384 contraction = 3 chunks of 128.

The HW Sin activation table is only accurate for |arg| <= ~pi, so arguments
are range-reduced first.  MOD/floor do not exist on DVE, but fp32->int
convert-on-write rounds-to-nearest (verified on HW for both DVE and GPSIMD),
so per feature f:
    t  = head * (lam_f/2pi)            (head = q or k, SBUF fp32)
    nf = int32(t + c_f)                 # GPSIMD/DVE tensor_scalar (mult, add)
    r  = t - nf                         # DVE scalar_tensor_tensor, fp32
    out = Sin(2pi * r + 2pi*c_f)        # ACT, per-partition bias, |arg|<=pi
      == sin(lam_f head + phi_f)        exactly (c_f = phi_f/2pi + bh lam/2pi)

Sharding: 8 cores = 4 batches x 2 query-halves (data-parallel, no
collectives).  Host rotates x per core so queries are always rows 0..511
(softmax over keys is order-invariant, so the rotated key order is fine).

Per-core pipeline (512 q x 1024 k):
  x -> xT (PE transposes) -> qT4/kT4 4x-replicated heads (PE matmul,
  partitions = 4 features x 32 u) -> heads to SBUF (ACT copies) ->
  range-reduce (GPSIMD p1 + DVE p2, chunk-pipelined) -> ACT Sin basis
  passes (per-partition bias; fp16 out) -> wco fold (DVE, 4x fp16) ->
  e = F^T G (PE, 3x2 accumulating matmuls per 128-query block) -> exp on ACT
  w/ accum_out row sums -> PE transpose (fp16) -> v = a^T @ x16 (PE) ->
  scale by 1/rowsum (DVE) -> DMA out.  Tails are staggered one block behind
  the e-matmuls; the last block's tail is split into key-halves to shorten
  the final serial chain.  A dummy Sin pins the sin activation table from
  t=0 (avoids a 1.3us mid-pipeline table swap), and ~14 filler matmuls keep
  the PE p-state ramped between the heads and the first e-matmul.
"""

import numpy as np

import concourse.bass as bass
import concourse.mybir as mybir
import concourse.tile as tile
from concourse import bacc
from concourse.bass import ds, ts

B, L, D, U = 4, 1024, 256, 32
NCORES = 8
HALVES = 2
LQ = L // HALVES                # 512 queries per core
NDC = D // 128                  # 2 contraction chunks for q/k projections
NJC = L // 128                  # 8 key chunks
NIB = LQ // 128                 # 4 query blocks
NFREQ = 6                       # fitted sine frequencies
NF = 2 * NFREQ                  # basis features (sin+cos per frequency)
NCH = NF // 4                   # 3 contraction chunks of 128 partitions
TWO_PI = float(2 * np.pi)

# Weighted nonlinear LSQ fit of tanh(s) ~= sum_j BCO[j] sin(LAM[j] s) over
# s in [-10, 10], weight exp(-s^2/11.5)+0.02 (s = q+k is approx N(0, 1.45^2)).
# End-to-end output rel err ~9e-4 (gate: 2e-3 local, 2e-2 harness).
LAM = [0.2781557257, 0.8391562854, 1.4111810023,
       2.0111781003, 2.7125102019, 3.6320141768]
BCO = [1.2355136439, 0.3262321042, 0.1278783401,
       0.0538655712, 0.0222520230, 0.0071252936]

F32 = mybir.dt.float32
F32R = mybir.dt.float32r
F16 = mybir.dt.float16
I32 = mybir.dt.int32
AF = mybir.ActivationFunctionType
ALU = mybir.AluOpType

# tabs columns: inv_per, c_q, c_k, bias_q (=2pi c_q), bias_k, wco
TINV, TCQ, TCK, TBQ, TBK, TWCO = range(6)


def build_kernel(nc: bass.Bass):
    xt_d = nc.dram_tensor("xt", [NDC, 128, L], F32R, kind="ExternalInput")
    x16_d = nc.dram_tensor("x16", [L, D], F16, kind="ExternalInput")
    wt4_d = nc.dram_tensor("wt4", [NDC, 128, 128], F32R, kind="ExternalInput")
    wx4_d = nc.dram_tensor("wx4", [NDC, 128, 128], F32R, kind="ExternalInput")
    tabs_d = nc.dram_tensor("tabs", [128, NCH, 6], F32, kind="ExternalInput")
    id16_d = nc.dram_tensor("id16", [128, 128], F16, kind="ExternalInput")
    out_d = nc.dram_tensor("out", [LQ, D], F32, kind="ExternalOutput")

    with tile.TileContext(nc) as tc:
        with tc.tile_pool(name="const", bufs=1) as cpool:
            x16_sb = cpool.tile([128, NJC, D], F16)
            xT_sb = cpool.tile([128, NDC, L], F32R)
            wt4_sb = cpool.tile([128, NDC, 128], F32R)
            wx4_sb = cpool.tile([128, NDC, 128], F32R)
            tabs_sb = cpool.tile([128, NCH, 6], F32)
            id16_sb = cpool.tile([128, 128], F16)
            qts_sb = cpool.tile([128, LQ], F32)
            kts_sb = cpool.tile([128, L], F32)
            msq_sb = cpool.tile([128, NCH, LQ], F32)
            msk_sb = cpool.tile([128, NCH, L], F32)
            ft_sb = cpool.tile([128, NCH, LQ], F16)
            gt_sb = cpool.tile([128, NCH, L], F16)
            sums_sb = cpool.tile([128, NIB, 2], F32)
            recip_sb = cpool.tile([128, NIB], F32)
            scr_sb = cpool.tile([128, 1], F16)

            # DMA order: weights first (gate the PE warm-up fillers and
            # heads), then xT in key-halves so kT4 n=0 can start early.
            xt_r = xt_d.ap().rearrange("c p j -> p c j")
            nc.sync.dma_start(
                wt4_sb[:], wt4_d.ap().rearrange("c p m -> p c m")
            )
            nc.sync.dma_start(
                wx4_sb[:], wx4_d.ap().rearrange("c p m -> p c m")
            )
            nc.sync.dma_start(xT_sb[:, :, ds(0, 512)], xt_r[:, :, 0:512])
            nc.sync.dma_start(xT_sb[:, :, ds(512, 512)], xt_r[:, :, 512:])
            nc.scalar.dma_start(tabs_sb[:], tabs_d.ap())
            nc.scalar.dma_start(id16_sb[:], id16_d.ap())
            nc.gpsimd.dma_start(
                x16_sb[:], x16_d.ap().rearrange("(c p) d -> p c d", p=128)
            )

            # Dummy Sin: pins the sin-containing activation table from the
            # start so no table swap lands on the basis-pass critical path.
            nc.scalar.activation(scr_sb[:], tabs_sb[:, 0, ds(0, 1)], AF.Sin)

            # ---- prologue: xT, q/k heads -> SBUF ----
            with (
                tc.tile_pool(name="pk", bufs=1, space="PSUM") as pk,
                tc.tile_pool(name="ptr", bufs=2, space="PSUM") as ptr,
                tc.tile_pool(name="pq", bufs=1, space="PSUM") as pq,
            ):
                kt_ps = pk.tile([128, L], F32)
                qt_ps = pq.tile([128, LQ], F32)
                # PE p-state warm-up: fillers gated only on wt4 (the first
                # DMA) so the tensor engine ramps before the heads arrive,
                # and interleaved with the heads to bridge to the e-matmuls.
                scr_ps = ptr.tile([128, 512], F32)

                def filler(count):
                    for dmy in range(count):
                        nc.tensor.matmul(
                            scr_ps[:, ds(0, 128)],
                            wt4_sb[:, 0, :],
                            wt4_sb[:, 0, :],
                            start=True,
                            stop=True,
                        )

                filler(8)
                for n in range(L // 512):
                    for dc in range(NDC):
                        nc.tensor.matmul(
                            kt_ps[:, ds(n * 512, 512)],
                            wx4_sb[:, dc, :],
                            xT_sb[:, dc, ds(n * 512, 512)],
                            start=(dc == 0),
                            stop=(dc == NDC - 1),
                        )
                    filler(2)
                for dc in range(NDC):
                    nc.tensor.matmul(
                        qt_ps[:],
                        wt4_sb[:, dc, :],
                        xT_sb[:, dc, ds(0, LQ)],
                        start=(dc == 0),
                        stop=(dc == NDC - 1),
                    )
                filler(10)
                nc.scalar.copy(kts_sb[:, ds(0, 512)], kt_ps[:, ds(0, 512)])
                nc.scalar.copy(kts_sb[:, ds(512, 512)], kt_ps[:, ds(512, 512)])
                nc.scalar.copy(qts_sb[:], qt_ps[:])

            # ---- chunk-pipelined range reduction + basis ----
            with (
                tc.tile_pool(name="nfq", bufs=2) as nfqp,
                tc.tile_pool(name="nfk", bufs=3) as nfkp,
                tc.tile_pool(name="spool", bufs=2) as spool,
            ):
                for c in range(NCH):
                    for nh in range(2):
                        nfk = nfkp.tile([128, 512], I32)
                        nc.gpsimd.tensor_scalar(
                            nfk[:],
                            kts_sb[:, ds(nh * 512, 512)],
                            tabs_sb[:, c, ds(TINV, 1)],
                            tabs_sb[:, c, ds(TCK, 1)],
                            ALU.mult,
                            ALU.add,
                        )
                        nc.vector.scalar_tensor_tensor(
                            msk_sb[:, c, ds(nh * 512, 512)],
                            kts_sb[:, ds(nh * 512, 512)],
                            tabs_sb[:, c, ds(TINV, 1)],
                            nfk[:],
                            ALU.mult,
                            ALU.subtract,
                        )
                    nfq = nfqp.tile([128, LQ], I32)
                    # q-side p1 on GPSIMD except the last chunk (balance)
                    p1q = nc.gpsimd if c < NCH - 1 else nc.vector
                    p1q.tensor_scalar(
                        nfq[:],
                        qts_sb[:],
                        tabs_sb[:, c, ds(TINV, 1)],
                        tabs_sb[:, c, ds(TCQ, 1)],
                        ALU.mult,
                        ALU.add,
                    )
                    nc.vector.scalar_tensor_tensor(
                        msq_sb[:, c, :],
                        qts_sb[:],
                        tabs_sb[:, c, ds(TINV, 1)],
                        nfq[:],
                        ALU.mult,
                        ALU.subtract,
                    )
                    ftm = spool.tile([128, LQ], F16, tag="ftm")
                    nc.scalar.activation(
                        ftm[:],
                        msq_sb[:, c, :],
                        AF.Sin,
                        bias=tabs_sb[:, c, ds(TBQ, 1)],
                        scale=TWO_PI,
                    )
                    nc.scalar.activation(
                        gt_sb[:, c, :],
                        msk_sb[:, c, :],
                        AF.Sin,
                        bias=tabs_sb[:, c, ds(TBK, 1)],
                        scale=TWO_PI,
                    )
                    nc.vector.tensor_scalar_mul(
                        ft_sb[:, c, :], ftm[:], tabs_sb[:, c, ds(TWCO, 1)]
                    )

                # ---- main loop over query blocks ----
                with (
                    tc.tile_pool(name="ppool", bufs=3) as ppool,
                    tc.tile_pool(name="atpool", bufs=2) as atpool,
                    tc.tile_pool(name="vpool", bufs=2) as vpool,
                    tc.tile_pool(name="pe", bufs=5, space="PSUM") as pe_e,
                    tc.tile_pool(name="pat", bufs=2, space="PSUM") as pe_at,
                    tc.tile_pool(name="pv", bufs=1, space="PSUM") as pe_v,
                ):
                    out_r = out_d.ap().rearrange("(ib p) d -> ib p d", p=128)
                    e_tiles = {}

                    def tail(ib, last=False):
                        p = ppool.tile([128, L], F16, tag="p")
                        at_ps = pe_at.tile([128, L], F16)
                        at_sb = atpool.tile([128, NJC, 128], F16, tag="at")
                        v_ps = pe_v.tile([128, D], F32)
                        hjc = NJC // 2
                        for h in (0, 1):
                            nc.scalar.activation(
                                p[:, ds(h * 512, 512)],
                                e_tiles[ib][h][:],
                                AF.Exp,
                                accum_out=sums_sb[:, ib, ds(h, 1)],
                            )
                            for jj in range(hjc):
                                jc = h * hjc + jj
                                nc.tensor.transpose(
                                    at_ps[:, ts(jc, 128)],
                                    p[:, ts(jc, 128)],
                                    id16_sb[:],
                                )
                            nc.vector.tensor_copy(
                                at_sb[:, h * hjc : (h + 1) * hjc, :],
                                at_ps[:, ds(h * 512, 512)],
                            )
                            for jj in range(hjc):
                                jc = h * hjc + jj
                                nc.tensor.matmul(
                                    v_ps[:],
                                    at_sb[:, jc, :],
                                    x16_sb[:, jc, :],
                                    start=(jc == 0),
                                    stop=(jc == NJC - 1),
                                )
                        nc.vector.tensor_tensor(
                            sums_sb[:, ib, ds(0, 1)],
                            sums_sb[:, ib, ds(0, 1)],
                            sums_sb[:, ib, ds(1, 1)],
                            ALU.add,
                        )
                        nc.vector.reciprocal(
                            recip_sb[:, ds(ib, 1)], sums_sb[:, ib, ds(0, 1)]
                        )
                        v_sb = vpool.tile([128, D], F32, tag="v")
                        nc.vector.tensor_scalar_mul(
                            v_sb[:], v_ps[:], recip_sb[:, ds(ib, 1)]
                        )
                        nc.sync.dma_start(out_r[ib], v_sb[:])

                    for ib in range(NIB):
                        e_h0 = pe_e.tile([128, 512], F32, tag="e")
                        e_h1 = pe_e.tile([128, 512], F32, tag="e")
                        e_h = [e_h0, e_h1]
                        e_tiles[ib] = e_h
                        for c in range(NCH):
                            for n in range(L // 512):
                                nc.tensor.matmul(
                                    e_h[n][:],
                                    ft_sb[:, c, ds(ib * 128, 128)],
                                    gt_sb[:, c, ds(n * 512, 512)],
                                    start=(c == 0),
                                    stop=(c == NCH - 1),
                                )
                        if ib >= 1:
                            tail(ib - 1)
                    tail(NIB - 1, last=True)

    return nc


_NC_CACHE: dict = {}


def get_compiled_nc():
    if "nc" not in _NC_CACHE:
        nc = bacc.Bacc("TRN2", target_bir_lowering=False, debug=False)
        build_kernel(nc)
        nc.compile()
        _NC_CACHE["nc"] = nc
    return _NC_CACHE["nc"]


def make_tables(bh, Wa):
    """Per-partition tables: partition p = 32*g + u holds feature f = 4*c + g
    (chunk c) for head u.  Feature f: frequency j = f//2; q-side phase
    phi = 0 (f even, sin) or pi/2 (f odd, cos); k-side phase is swapped so
    sum_f F*G telescopes to sum_j b_j sin(lam_j (a+b))."""
    tabs = np.zeros((128, NCH, 6), np.float64)
    for c in range(NCH):
        for g in range(4):
            f = 4 * c + g
            j = f // 2
            lam = LAM[j]
            phi_q = 0.0 if f % 2 == 0 else np.pi / 2
            phi_k = np.pi / 2 if f % 2 == 0 else 0.0
            for u in range(U):
                p = 32 * g + u
                cq = phi_q / (2 * np.pi) + bh[u] * lam / (2 * np.pi)
                ck = phi_k / (2 * np.pi)
                tabs[p, c, TINV] = lam / (2 * np.pi)
                tabs[p, c, TCQ] = cq
                tabs[p, c, TCK] = ck
                tabs[p, c, TBQ] = 2 * np.pi * cq
                tabs[p, c, TBK] = 2 * np.pi * ck
                tabs[p, c, TWCO] = Wa[u, 0] * BCO[j]
    return tabs.astype(np.float32)


def make_in_maps(inputs_np, Wt, Wx, bh, Wa):
    wt4 = np.zeros((NDC, 128, 128), np.float32)
    wx4 = np.zeros((NDC, 128, 128), np.float32)
    for dc in range(NDC):
        wt4[dc] = np.tile(Wt[dc * 128 : (dc + 1) * 128], (1, 4))
        wx4[dc] = np.tile(Wx[dc * 128 : (dc + 1) * 128], (1, 4))
    tabs = make_tables(bh, Wa)
    id16 = np.eye(128, dtype=np.float16)
    in_maps = []
    for core in range(NCORES):
        b, half = divmod(core, HALVES)
        xr = np.roll(inputs_np[b], -half * LQ, axis=0)
        in_maps.append(
            {
                "xt": np.ascontiguousarray(xr.T.reshape(NDC, 128, L)),
                "x16": np.ascontiguousarray(xr.astype(np.float16)),
                "wt4": wt4,
                "wx4": wx4,
                "tabs": tabs,
                "id16": id16,
            }
        )
    return in_maps


def kernel(**inputs) -> np.ndarray:
    x = np.asarray(inputs["inputs"], dtype=np.float32)
    Wt = np.ascontiguousarray(np.asarray(inputs["Wt"], np.float32))
    Wx = np.ascontiguousarray(np.asarray(inputs["Wx"], np.float32))
    bh = np.asarray(inputs["bh"], np.float32)
    Wa = np.asarray(inputs["Wa"], np.float32)

    from concourse.bass_utils import run_bass_kernel_spmd

    nc = get_compiled_nc()
    in_maps = make_in_maps(x, Wt, Wx, bh, Wa)
    res = run_bass_kernel_spmd(nc, in_maps, list(range(NCORES)))
    kernel._last_results = res  # type: ignore[attr-defined]

    out = np.empty((B, L, D), np.float32)
    for core in range(NCORES):
        b, half = divmod(core, HALVES)
        out[b, half * LQ : (half + 1) * LQ] = res.results[core]["out"]
    return out


# revision 26
# speedup vs baseline: 1.0483x; 1.0106x over previous
"""Bahdanau additive-attention pooling for Trainium2 (Bass/Tile).

Reference math (per batch):
    q = x @ Wt; k = x @ Wx                                  [L, U]
    e[i,j] = sum_u Wa[u] * tanh(q[i,u] + k[j,u] + bh[u])    (+ ba, dropped --
                                                             softmax shift-inv)
    v = softmax_j(e) @ x                                    [L, D]

Key trick: tanh(a+b) is approximated by a short sine expansion with FITTED
frequencies (weighted nonlinear LSQ over the data distribution of s = a+b):
    tanh(s) ~= sum_j b_j sin(lam_j s),   j = 1..6
which is SEPARABLE:
    sin(lam(a+b)) = sin(lam a)cos(lam b) + cos(lam a)sin(lam b)
so the [L, L, U] tanh volume (the baseline bottleneck: ~131us of ScalarE at
1 elem/cycle/lane) collapses into a rank-12-per-u PE matmul:
    e[i,j] = sum_{u,f} F[(u,f), i] * G[(u,f), j]
    F = Wa_u b_j sin(lam_f q_iu + phi_f),  G = sin(lam_f k_ju + psi_f)
with NF = 12 features -> 32*12 = 384 contraction = 3 chunks of 128.

The HW Sin activation table is only accurate for |arg| <= ~pi, so arguments
are range-reduced first.  MOD/floor do not exist on DVE, but fp32->int
convert-on-write rounds-to-nearest (verified on HW for both DVE and GPSIMD),
so per feature f:
    t  = head * (lam_f/2pi)            (head = q or k, SBUF fp32)
    nf = int32(t + c_f)                 # GPSIMD/DVE tensor_scalar (mult, add)
    r  = t - nf                         # DVE scalar_tensor_tensor, fp32
    out = Sin(2pi * r + 2pi*c_f)        # ACT, per-partition bias, |arg|<=pi
      == sin(lam_f head + phi_f)        exactly (c_f = phi_f/2pi + bh lam/2pi)

Sharding: 8 cores = 4 batches x 2 query-halves (data-parallel, no
collectives).  Host rotates x per core so queries are always rows 0..511
(softmax over keys is order-invariant, so the rotated key order is fine).

Per-core pipeline (512 q x 1024 k):
  x -> xT (PE transposes) -> qT4/kT4 4x-replicated heads (PE matmul,
  partitions = 4 features x 32 u) -> heads to SBUF (ACT copies) ->
  range-reduce (GPSIMD p1 + DVE p2, chunk-pipelined) -> ACT Sin basis
  passes (per-partition bias; fp16 out) -> wco fold (DVE, 4x fp16) ->
  e = F^T G (PE, 3x2 accumulating matmuls per 128-query block) -> exp on ACT
  w/ accum_out row sums -> PE transpose (fp16) -> v = a^T @ x16 (PE) ->
  scale by 1/rowsum (DVE) -> DMA out.  Tails are staggered one block behind
  the e-matmuls; the last block's tail is split into key-halves to shorten
  the final serial chain.  A dummy Sin pins the sin activation table from
  t=0 (avoids a 1.3us mid-pipeline table swap), and ~14 filler matmuls keep
  the PE p-state ramped between the heads and the first e-matmul.
"""

import numpy as np

import concourse.bass as bass
import concourse.mybir as mybir
import concourse.tile as tile
from concourse import bacc
from concourse.bass import ds, ts

B, L, D, U = 4, 1024, 256, 32
NCORES = 8
HALVES = 2
LQ = L // HALVES                # 512 queries per core
NDC = D // 128                  # 2 contraction chunks for q/k projections
NJC = L // 128                  # 8 key chunks
NIB = LQ // 128                 # 4 query blocks
NFREQ = 6                       # fitted sine frequencies
NF = 2 * NFREQ                  # basis features (sin+cos per frequency)
NCH = NF // 4                   # 3 contraction chunks of 128 partitions
TWO_PI = float(2 * np.pi)

# Weighted nonlinear LSQ fit of tanh(s) ~= sum_j BCO[j] sin(LAM[j] s) over
# s in [-10, 10], weight exp(-s^2/11.5)+0.02 (s = q+k is approx N(0, 1.45^2)).
# End-to-end output rel err ~9e-4 (gate: 2e-3 local, 2e-2 harness).
LAM = [0.2781557257, 0.8391562854, 1.4111810023,
       2.0111781003, 2.7125102019, 3.6320141768]
BCO = [1.2355136439, 0.3262321042, 0.1278783401,
       0.0538655712, 0.0222520230, 0.0071252936]

F32 = mybir.dt.float32
F32R = mybir.dt.float32r
F16 = mybir.dt.float16
I32 = mybir.dt.int32
AF = mybir.ActivationFunctionType
ALU = mybir.AluOpType

# tabs columns: inv_per, c_q, c_k, bias_q (=2pi c_q), bias_k, wco
TINV, TCQ, TCK, TBQ, TBK, TWCO = range(6)


def build_kernel(nc: bass.Bass):
    xt_d = nc.dram_tensor("xt", [NDC, 128, L], F32R, kind="ExternalInput")
    x16_d = nc.dram_tensor("x16", [L, D], F16, kind="ExternalInput")
    wt4_d = nc.dram_tensor("wt4", [NDC, 128, 128], F32R, kind="ExternalInput")
    wx4_d = nc.dram_tensor("wx4", [NDC, 128, 128], F32R, kind="ExternalInput")
    tabs_d = nc.dram_tensor("tabs", [128, NCH, 6], F32, kind="ExternalInput")
    id16_d = nc.dram_tensor("id16", [128, 128], F16, kind="ExternalInput")
    out_d = nc.dram_tensor("out", [LQ, D], F32, kind="ExternalOutput")

    with tile.TileContext(nc) as tc:
        with tc.tile_pool(name="const", bufs=1) as cpool:
            x16_sb = cpool.tile([128, NJC, D], F16)
            xT_sb = cpool.tile([128, NDC, L], F32R)
            wt4_sb = cpool.tile([128, NDC, 128], F32R)
            wx4_sb = cpool.tile([128, NDC, 128], F32R)
            tabs_sb = cpool.tile([128, NCH, 6], F32)
            id16_sb = cpool.tile([128, 128], F16)
            qts_sb = cpool.tile([128, LQ], F32)
            kts_sb = cpool.tile([128, L], F32)
            msq_sb = cpool.tile([128, NCH, LQ], F32)
            msk_sb = cpool.tile([128, NCH, L], F32)
            ft_sb = cpool.tile([128, NCH, LQ], F16)
            gt_sb = cpool.tile([128, NCH, L], F16)
            sums_sb = cpool.tile([128, NIB, 2], F32)
            recip_sb = cpool.tile([128, NIB], F32)
            scr_sb = cpool.tile([128, 1], F16)

            # DMA order: weights first (gate the PE warm-up fillers and
            # heads), then xT in key-halves so kT4 n=0 can start early.
            xt_r = xt_d.ap().rearrange("c p j -> p c j")
            nc.sync.dma_start(
                wt4_sb[:], wt4_d.ap().rearrange("c p m -> p c m")
            )
            nc.sync.dma_start(
                wx4_sb[:], wx4_d.ap().rearrange("c p m -> p c m")
            )
            nc.sync.dma_start(xT_sb[:, :, ds(0, 512)], xt_r[:, :, 0:512])
            nc.sync.dma_start(xT_sb[:, :, ds(512, 512)], xt_r[:, :, 512:])
            nc.scalar.dma_start(tabs_sb[:], tabs_d.ap())
            nc.scalar.dma_start(id16_sb[:], id16_d.ap())
            nc.gpsimd.dma_start(
                x16_sb[:], x16_d.ap().rearrange("(c p) d -> p c d", p=128)
            )

            # Dummy Sin: pins the sin-containing activation table from the
            # start so no table swap lands on the basis-pass critical path.
            nc.scalar.activation(scr_sb[:], tabs_sb[:, 0, ds(0, 1)], AF.Sin)

            # ---- prologue: xT, q/k heads -> SBUF ----
            with (
                tc.tile_pool(name="pk", bufs=1, space="PSUM") as pk,
                tc.tile_pool(name="ptr", bufs=2, space="PSUM") as ptr,
                tc.tile_pool(name="pq", bufs=1, space="PSUM") as pq,
            ):
                kt_ps = pk.tile([128, L], F32)
                qt_ps = pq.tile([128, LQ], F32)
                # PE p-state warm-up: fillers gated only on wt4 (the first
                # DMA) so the tensor engine ramps before the heads arrive,
                # and interleaved with the heads to bridge to the e-matmuls.
                scr_ps = ptr.tile([128, 512], F32)

                def filler(count):
                    for dmy in range(count):
                        nc.tensor.matmul(
                            scr_ps[:, ds(0, 128)],
                            wt4_sb[:, 0, :],
                            wt4_sb[:, 0, :],
                            start=True,
                            stop=True,
                        )

                filler(8)
                for n in range(L // 512):
                    for dc in range(NDC):
                        nc.tensor.matmul(
                            kt_ps[:, ds(n * 512, 512)],
                            wx4_sb[:, dc, :],
                            xT_sb[:, dc, ds(n * 512, 512)],
                            start=(dc == 0),
                            stop=(dc == NDC - 1),
                        )
                    filler(2)
                for dc in range(NDC):
                    nc.tensor.matmul(
                        qt_ps[:],
                        wt4_sb[:, dc, :],
                        xT_sb[:, dc, ds(0, LQ)],
                        start=(dc == 0),
                        stop=(dc == NDC - 1),
                    )
                filler(10)
                nc.scalar.copy(kts_sb[:, ds(0, 512)], kt_ps[:, ds(0, 512)])
                nc.scalar.copy(kts_sb[:, ds(512, 512)], kt_ps[:, ds(512, 512)])
                nc.scalar.copy(qts_sb[:], qt_ps[:])

            # ---- chunk-pipelined range reduction + basis ----
            with (
                tc.tile_pool(name="nfq", bufs=2) as nfqp,
                tc.tile_pool(name="nfk", bufs=2) as nfkp,
                tc.tile_pool(name="spool", bufs=2) as spool,
            ):
                for c in range(NCH):
                    nfk = nfkp.tile([128, L], I32)
                    nc.gpsimd.tensor_scalar(
                        nfk[:],
                        kts_sb[:],
                        tabs_sb[:, c, ds(TINV, 1)],
                        tabs_sb[:, c, ds(TCK, 1)],
                        ALU.mult,
                        ALU.add,
                    )
                    nc.vector.scalar_tensor_tensor(
                        msk_sb[:, c, :],
                        kts_sb[:],
                        tabs_sb[:, c, ds(TINV, 1)],
                        nfk[:],
                        ALU.mult,
                        ALU.subtract,
                    )
                    nfq = nfqp.tile([128, LQ], I32)
                    # q-side p1 on GPSIMD except the last chunk (balance)
                    p1q = nc.gpsimd if c < NCH - 1 else nc.vector
                    p1q.tensor_scalar(
                        nfq[:],
                        qts_sb[:],
                        tabs_sb[:, c, ds(TINV, 1)],
                        tabs_sb[:, c, ds(TCQ, 1)],
                        ALU.mult,
                        ALU.add,
                    )
                    nc.vector.scalar_tensor_tensor(
                        msq_sb[:, c, :],
                        qts_sb[:],
                        tabs_sb[:, c, ds(TINV, 1)],
                        nfq[:],
                        ALU.mult,
                        ALU.subtract,
                    )
                    ftm = spool.tile([128, LQ], F16, tag="ftm")
                    nc.scalar.activation(
                        ftm[:],
                        msq_sb[:, c, :],
                        AF.Sin,
                        bias=tabs_sb[:, c, ds(TBQ, 1)],
                        scale=TWO_PI,
                    )
                    nc.scalar.activation(
                        gt_sb[:, c, :],
                        msk_sb[:, c, :],
                        AF.Sin,
                        bias=tabs_sb[:, c, ds(TBK, 1)],
                        scale=TWO_PI,
                    )
                    nc.vector.tensor_scalar_mul(
                        ft_sb[:, c, :], ftm[:], tabs_sb[:, c, ds(TWCO, 1)]
                    )

                # ---- main loop over query blocks ----
                with (
                    tc.tile_pool(name="ppool", bufs=3) as ppool,
                    tc.tile_pool(name="atpool", bufs=2) as atpool,
                    tc.tile_pool(name="vpool", bufs=2) as vpool,
                    tc.tile_pool(name="pe", bufs=5, space="PSUM") as pe_e,
                    tc.tile_pool(name="pat", bufs=2, space="PSUM") as pe_at,
                    tc.tile_pool(name="pv", bufs=1, space="PSUM") as pe_v,
                ):
                    out_r = out_d.ap().rearrange("(ib p) d -> ib p d", p=128)
                    e_tiles = {}

                    def tail(ib, last=False):
                        p = ppool.tile([128, L], F16, tag="p")
                        at_ps = pe_at.tile([128, L], F16)
                        at_sb = atpool.tile([128, NJC, 128], F16, tag="at")
                        v_ps = pe_v.tile([128, D], F32)
                        hjc = NJC // 2
                        for h in (0, 1):
                            nc.scalar.activation(
                                p[:, ds(h * 512, 512)],
                                e_tiles[ib][h][:],
                                AF.Exp,
                                accum_out=sums_sb[:, ib, ds(h, 1)],
                            )
                            for jj in range(hjc):
                                jc = h * hjc + jj
                                nc.tensor.transpose(
                                    at_ps[:, ts(jc, 128)],
                                    p[:, ts(jc, 128)],
                                    id16_sb[:],
                                )
                            nc.vector.tensor_copy(
                                at_sb[:, h * hjc : (h + 1) * hjc, :],
                                at_ps[:, ds(h * 512, 512)],
                            )
                            for jj in range(hjc):
                                jc = h * hjc + jj
                                nc.tensor.matmul(
                                    v_ps[:],
                                    at_sb[:, jc, :],
                                    x16_sb[:, jc, :],
                                    start=(jc == 0),
                                    stop=(jc == NJC - 1),
                                )
                        nc.vector.tensor_tensor(
                            sums_sb[:, ib, ds(0, 1)],
                            sums_sb[:, ib, ds(0, 1)],
                            sums_sb[:, ib, ds(1, 1)],
                            ALU.add,
                        )
                        nc.vector.reciprocal(
                            recip_sb[:, ds(ib, 1)], sums_sb[:, ib, ds(0, 1)]
                        )
                        v_sb = vpool.tile([128, D], F32, tag="v")
                        nc.vector.tensor_scalar_mul(
                            v_sb[:], v_ps[:], recip_sb[:, ds(ib, 1)]
                        )
                        nc.sync.dma_start(out_r[ib], v_sb[:])

                    for ib in range(NIB):
                        e_h0 = pe_e.tile([128, 512], F32, tag="e")
                        e_h1 = pe_e.tile([128, 512], F32, tag="e")
                        e_h = [e_h0, e_h1]
                        e_tiles[ib] = e_h
                        for c in range(NCH):
                            for n in range(L // 512):
                                nc.tensor.matmul(
                                    e_h[n][:],
                                    ft_sb[:, c, ds(ib * 128, 128)],
                                    gt_sb[:, c, ds(n * 512, 512)],
                                    start=(c == 0),
                                    stop=(c == NCH - 1),
                                )
                        if ib >= 1:
                            tail(ib - 1)
                    tail(NIB - 1, last=True)

    return nc


_NC_CACHE: dict = {}


def get_compiled_nc():
    if "nc" not in _NC_CACHE:
        nc = bacc.Bacc("TRN2", target_bir_lowering=False, debug=False)
        build_kernel(nc)
        nc.compile()
        _NC_CACHE["nc"] = nc
    return _NC_CACHE["nc"]


def make_tables(bh, Wa):
    """Per-partition tables: partition p = 32*g + u holds feature f = 4*c + g
    (chunk c) for head u.  Feature f: frequency j = f//2; q-side phase
    phi = 0 (f even, sin) or pi/2 (f odd, cos); k-side phase is swapped so
    sum_f F*G telescopes to sum_j b_j sin(lam_j (a+b))."""
    tabs = np.zeros((128, NCH, 6), np.float64)
    for c in range(NCH):
        for g in range(4):
            f = 4 * c + g
            j = f // 2
            lam = LAM[j]
            phi_q = 0.0 if f % 2 == 0 else np.pi / 2
            phi_k = np.pi / 2 if f % 2 == 0 else 0.0
            for u in range(U):
                p = 32 * g + u
                cq = phi_q / (2 * np.pi) + bh[u] * lam / (2 * np.pi)
                ck = phi_k / (2 * np.pi)
                tabs[p, c, TINV] = lam / (2 * np.pi)
                tabs[p, c, TCQ] = cq
                tabs[p, c, TCK] = ck
                tabs[p, c, TBQ] = 2 * np.pi * cq
                tabs[p, c, TBK] = 2 * np.pi * ck
                tabs[p, c, TWCO] = Wa[u, 0] * BCO[j]
    return tabs.astype(np.float32)


def make_in_maps(inputs_np, Wt, Wx, bh, Wa):
    wt4 = np.zeros((NDC, 128, 128), np.float32)
    wx4 = np.zeros((NDC, 128, 128), np.float32)
    for dc in range(NDC):
        wt4[dc] = np.tile(Wt[dc * 128 : (dc + 1) * 128], (1, 4))
        wx4[dc] = np.tile(Wx[dc * 128 : (dc + 1) * 128], (1, 4))
    tabs = make_tables(bh, Wa)
    id16 = np.eye(128, dtype=np.float16)
    in_maps = []
    for core in range(NCORES):
        b, half = divmod(core, HALVES)
        xr = np.roll(inputs_np[b], -half * LQ, axis=0)
        in_maps.append(
            {
                "xt": np.ascontiguousarray(xr.T.reshape(NDC, 128, L)),
                "x16": np.ascontiguousarray(xr.astype(np.float16)),
                "wt4": wt4,
                "wx4": wx4,
                "tabs": tabs,
                "id16": id16,
            }
        )
    return in_maps


def kernel(**inputs) -> np.ndarray:
    x = np.asarray(inputs["inputs"], dtype=np.float32)
    Wt = np.ascontiguousarray(np.asarray(inputs["Wt"], np.float32))
    Wx = np.ascontiguousarray(np.asarray(inputs["Wx"], np.float32))
    bh = np.asarray(inputs["bh"], np.float32)
    Wa = np.asarray(inputs["Wa"], np.float32)

    from concourse.bass_utils import run_bass_kernel_spmd

    nc = get_compiled_nc()
    in_maps = make_in_maps(x, Wt, Wx, bh, Wa)
    res = run_bass_kernel_spmd(nc, in_maps, list(range(NCORES)))
    kernel._last_results = res  # type: ignore[attr-defined]

    out = np.empty((B, L, D), np.float32)
    for core in range(NCORES):
        b, half = divmod(core, HALVES)
        out[b, half * LQ : (half + 1) * LQ] = res.results[core]["out"]
    return out
